# revision 1
# baseline (speedup 1.0000x reference)
"""Multi-head causal attention on 8 Trainium2 NeuronCores.

nn_MultiHeadAttention_37933151158277: x[2,2048,2048] f32, causal mask,
W_qkv[6144,2048], W_o[2048,2048]. Tensor-parallel over heads (2 per
core), per the sharding hint: qkv_proj output and W_o input are split
along the head dimension; x is replicated. Each core:

  phase 1 - QKV projection. Host supplies x^T [D, T] and per-core
      weight slices pre-transposed, so Q^T/K^T land as [d_k=128, tok]
      and V as [tok, d_k] with zero on-device transposes. Weights are
      SBUF-resident; x^T streams through in [128, 512] tiles feeding 8
      accumulating PSUM banks (4 Q/K e-tiles + 4 V token-tiles).
  phase 2 - attention per (batch, head). Scores are computed
      transposed: S^T[k, q] = K^T_tile.T @ Q^T (contraction over d), so
      the P @ V matmul can consume exp(S^T) directly with V tiles as
      the stationary operand. No max-subtraction (scores are O(1) by
      construction, exp cannot overflow); softmax denominator is
      accumulated in two parallel chains (DVE + GPSIMD, merged once at
      the q-block tail to halve the serial add depth) + one
      ones-vector matmul, then
      1/d is broadcast across partitions with a K=1 matmul (d and the
      broadcast share one fast-churning PSUM tile so q-block tails
      never block the next block's score matmuls). The two
      head-streams of a batch are emitted entry-interleaved so the
      in-order PE queue never stalls on one stream's exp; exp runs on
      2-k-tile PSUM groups to amortize ACT call overhead.
  phase 3 - partial out-projection y_c = attn_out @ W_o[:, cols]^T,
      emitted one q-block behind attention so its dependencies are
      ready when the PE reaches them.

Host: y = sum_c y_c (the unshard of the head-parallel partial sums).

All matmuls run float32r (full PE rate; end-to-end error ~2.5e-4
scale-relative absmax vs the fp32 reference). The mask is analyzed
block-wise at trace time: fully-masked blocks are skipped, fully-valid
blocks skip the mask multiply, mixed blocks get a (content-deduped)
DMA'd mask-tile multiply - for the causal mask this yields the optimal
lower-triangular schedule with a single shared 128x128 triangle tile.

Measured: ~467 us on hardware per core (8 cores SPMD), rel err 2.5e-4.
"""
import sys
if '/opt/trn_rl_repo' not in sys.path:
    sys.path.insert(0, '/opt/trn_rl_repo')

import numpy as np

B, S, D = 2, 2048, 2048
H, DK = 16, 128
NCORES = 8
HPC = H // NCORES            # heads per core
T = B * S                    # tokens
QB = 512                     # q-block width (free dim of S^T / PV matmuls)
NKT = S // 128               # k tiles per batch (16)
NQB = S // QB                # q blocks per batch (4)
NCH = T // QB                # token chunks (8)
NDT = D // 128               # d_model tiles (16)
SGRP = 1                     # k-tiles per s-psum group

_cache = {}


def _analyze_mask(m2):
    """m2: [S, S] bool, m2[q, k]. Returns blocks[qb] = list of entries
    (j, q0c, mm0, mm1) ascending j:
      q0c: first q col (within block) to compute, mm0..mm1: mask-mul range
      (None if block fully valid over [q0c, QB)).
    """
    blocks = []
    for qb in range(NQB):
        entries = []
        for j in range(NKT):
            blk = m2[qb * QB:(qb + 1) * QB, j * 128:(j + 1) * 128]
            col_any = blk.any(axis=1)
            if not col_any.any():
                continue
            col_all = blk.all(axis=1)
            q0 = int(np.argmax(col_any))
            # q1: start of the trailing fully-valid run
            rev = col_all[::-1]
            run = int(np.argmin(rev)) if not rev.all() else QB
            q1 = QB - run
            if q1 <= q0:
                entries.append((j, q0, None, None))
            else:
                entries.append((j, q0, q0, q1))
        if entries:
            qmin = min(e[1] for e in entries)
            j, q0, m0, m1 = entries[0]
            if q0 > qmin:
                # first entry must cover every column later entries write
                entries[0] = (j, qmin, qmin, m1 if m1 is not None else q0)
        blocks.append(entries)
    return blocks


def _build(mask_bool):
    from contextlib import ExitStack
    import concourse.bass as bass
    import concourse.tile as tile
    from concourse import bacc, mybir

    f32 = mybir.dt.float32
    f32r = mybir.dt.float32r
    EXP = mybir.ActivationFunctionType.Exp
    scale = 1.0 / np.sqrt(DK)

    m2 = mask_bool
    blocks = _analyze_mask(m2)

    nc = bacc.Bacc("TRN2", target_bir_lowering=False, debug=False)
    xt_d = nc.dram_tensor("xt", [D, T], f32r, kind="ExternalInput")
    wqk_d = nc.dram_tensor("wqk", [D, 4 * 128], f32r, kind="ExternalInput")
    wv_d = nc.dram_tensor("wv", [D, 2 * 128], f32r, kind="ExternalInput")
    wo_d = nc.dram_tensor("wo", [2 * 128, D], f32r, kind="ExternalInput")
    mt_d = nc.dram_tensor("mt", [S, S], f32r, kind="ExternalInput")
    y_d = nc.dram_tensor("y", [T, D], f32, kind="ExternalOutput")
    import os as _os
    dump = bool(_os.environ.get("KERNEL_DUMP"))
    if dump:
        qk_dump = nc.dram_tensor("qk_dump", [512, T], f32r, kind="ExternalOutput")
        v_dump = nc.dram_tensor("v_dump", [128, (T // 128) * 256], f32r,
                                kind="ExternalOutput")
        at_dump = nc.dram_tensor("at_dump", [256, T], f32r, kind="ExternalOutput")

    with tile.TileContext(nc) as tc:
        with ExitStack() as stack:
            stack.enter_context(
                nc.allow_low_precision(reason="float32r matmul inputs"))
            qkt_pool = stack.enter_context(tc.tile_pool(name="qkt", bufs=1))
            v_pool = stack.enter_context(tc.tile_pool(name="vsb", bufs=1))
            att_pool = stack.enter_context(tc.tile_pool(name="att", bufs=1))
            cst_pool = stack.enter_context(tc.tile_pool(name="cst", bufs=1))

            # persistent SBUF
            qt_sb = [qkt_pool.tile([128, T], f32r, tag=f"qt{h}", name=f"qt{h}")
                     for h in range(HPC)]
            kt_sb = [qkt_pool.tile([128, T], f32r, tag=f"kt{h}", name=f"kt{h}")
                     for h in range(HPC)]
            v_sb = v_pool.tile([128, (T // 128) * 256], f32r, tag="v")
            at_sb = [att_pool.tile([128, T], f32r, tag=f"at{h}", name=f"at{h}")
                     for h in range(HPC)]

            wo_pool = stack.enter_context(tc.tile_pool(name="wo", bufs=1))
            wo_sb = []
            for h in range(HPC):
                wt = wo_pool.tile([128, D], f32r, tag=f"wo{h}", name=f"wo{h}")
                nc.sync.dma_start(wt[:], wo_d.ap()[h * 128:(h + 1) * 128, :])
                wo_sb.append(wt)

            ones_f = cst_pool.tile([128, 128], f32, tag="ones_f")
            nc.vector.memset(ones_f[:], 1.0)
            ones_col = cst_pool.tile([128, 1], f32r, tag="ones_c")
            nc.scalar.copy(ones_col[:], ones_f[:, 0:1])
            ones_row = cst_pool.tile([1, 128], f32r, tag="ones_r")
            nc.scalar.copy(ones_row[:], ones_f[0:1, :])

            # ---------------- phase 1: QKV projection ----------------
            with ExitStack() as p1:
                wqk_pool = p1.enter_context(tc.tile_pool(name="wqk", bufs=1))
                wv_pool = p1.enter_context(tc.tile_pool(name="wv", bufs=1))
                xt_pool = p1.enter_context(tc.tile_pool(name="xt", bufs=5))
                qk_ps_pool = p1.enter_context(
                    tc.tile_pool(name="ps_qk", bufs=4, space="PSUM"))
                v_ps_pool = p1.enter_context(
                    tc.tile_pool(name="ps_v", bufs=4, space="PSUM"))

                wqk_sb = []
                wv_sb = []
                for kd in range(NDT):
                    wq = wqk_pool.tile([128, 512], f32r, tag=f"wqk{kd}")
                    nc.sync.dma_start(wq[:], wqk_d.ap()[kd * 128:(kd + 1) * 128, :])
                    wqk_sb.append(wq)
                    wv_t = wv_pool.tile([128, 256], f32r, tag=f"wv{kd}")
                    nc.sync.dma_start(wv_t[:], wv_d.ap()[kd * 128:(kd + 1) * 128, :])
                    wv_sb.append(wv_t)

                for c in range(NCH):
                    qk_ps = [qk_ps_pool.tile([128, 512], f32, tag="qk", name="qkps")
                             for _ in range(4)]
                    v_ps = [v_ps_pool.tile([128, 256], f32, tag="v", name="vps")
                            for _ in range(4)]
                    for kd in range(NDT):
                        xt_t = xt_pool.tile([128, 512], f32r, tag="xt")
                        nc.sync.dma_start(
                            xt_t[:], xt_d.ap()[kd * 128:(kd + 1) * 128,
                                               c * 512:(c + 1) * 512])
                        st, sp = kd == 0, kd == NDT - 1
                        for e in range(4):
                            nc.tensor.matmul(
                                qk_ps[e][:], wqk_sb[kd][:, e * 128:(e + 1) * 128],
                                xt_t[:], start=st, stop=sp)
                        for tl in range(4):
                            nc.tensor.matmul(
                                v_ps[tl][:],
                                xt_t[:, tl * 128:(tl + 1) * 128],
                                wv_sb[kd][:], start=st, stop=sp)
                    dsts = [qt_sb[0], qt_sb[1], kt_sb[0], kt_sb[1]]
                    for e in range(4):
                        nc.vector.tensor_copy(
                            dsts[e][:, c * 512:(c + 1) * 512], qk_ps[e][:])
                    for tl in range(4):
                        tok = c * 4 + tl
                        nc.scalar.copy(
                            v_sb[:, tok * 256:(tok + 1) * 256], v_ps[tl][:])

            # ---------------- phase 2 + 3: attention + projection ----------------
            with ExitStack() as p2:
                e_pool = p2.enter_context(tc.tile_pool(name="e", bufs=4))
                acc_pool = p2.enter_context(tc.tile_pool(name="acc", bufs=2))
                rcp_pool = p2.enter_context(tc.tile_pool(name="rcp", bufs=2))
                b_pool = p2.enter_context(tc.tile_pool(name="bsb", bufs=2))
                msk_pool = p2.enter_context(tc.tile_pool(name="msk", bufs=1))
                ysb_pool = p2.enter_context(tc.tile_pool(name="ysb", bufs=4))
                s_ps_pool = p2.enter_context(
                    tc.tile_pool(name="ps_s", bufs=2, space="PSUM"))
                o_ps_pool = p2.enter_context(
                    tc.tile_pool(name="ps_o", bufs=2, space="PSUM"))
                y_ps_pool = p2.enter_context(
                    tc.tile_pool(name="ps_y", bufs=2, space="PSUM"))

                # mask tile cache keyed by block content
                mask_tiles = {}

                def mask_tile(j, qb, m0, m1):
                    key = m2[qb * QB + m0:qb * QB + m1,
                             j * 128:(j + 1) * 128].tobytes()
                    t = mask_tiles.get(key)
                    if t is None:
                        t = msk_pool.tile([128, QB], f32r, name=f"mask{len(mask_tiles)}",
                                          tag=f"m{len(mask_tiles)}")
                        nc.sync.dma_start(
                            t[:, 0:m1 - m0],
                            mt_d.ap()[j * 128:(j + 1) * 128,
                                      qb * QB + m0:qb * QB + m1])
                        mask_tiles[key] = t
                    return t

                # Attention: the two head-streams of a batch are emitted
                # entry-interleaved (h0/h1 alternating per k-tile) so the PE
                # queue never blocks on one stream's exp; projection tiles
                # are emitted one q-block behind the attention that produces
                # their inputs, so their dependencies are ready when the
                # in-order PE queue reaches them.
                class QbStream:
                    def __init__(self, b, h, qb):
                        self.b, self.h, self.qb = b, h, qb
                        self.tb = b * S
                        self.entries = blocks[qb]
                        self.ne = len(self.entries)
                        self.accA = acc_pool.tile([128, QB], f32r,
                                                  tag="accA", name="accA")
                        self.accB = acc_pool.tile([128, QB], f32r,
                                                  tag="accB", name="accB")
                        self.startA = None  # leftmost initialized col
                        self.startB = None
                        self.o_ps = o_ps_pool.tile([128, QB], f32, tag="o",
                                                   name="ops")
                        self.qcol = self.tb + qb * QB
                        self.pend = None
                        self.gi = 0

                    def s_and_exp(self, grp):
                        s_ps = s_ps_pool.tile([128, 2 * QB], f32, tag="s",
                                              name="sps")
                        for idx, (j, q0c, m0, m1) in enumerate(grp):
                            nc.tensor.matmul(
                                s_ps[:, idx * QB + q0c:(idx + 1) * QB],
                                kt_sb[self.h][:, self.tb + j * 128:
                                              self.tb + (j + 1) * 128],
                                qt_sb[self.h][:, self.qcol + q0c:
                                              self.qcol + QB],
                                start=True, stop=True)
                        e_sb = e_pool.tile([128, 2 * QB], f32r, tag="e",
                                           name="esb")
                        if len(grp) == 2 and all(e[1] == 0 for e in grp):
                            nc.scalar.activation(e_sb[:], s_ps[:], EXP,
                                                 scale=scale)
                        else:
                            for idx, (j, q0c, m0, m1) in enumerate(grp):
                                lo = idx * QB + q0c
                                hi = (idx + 1) * QB
                                nc.scalar.activation(
                                    e_sb[:, lo:hi], s_ps[:, lo:hi], EXP,
                                    scale=scale)
                        for idx, (j, q0c, m0, m1) in enumerate(grp):
                            if m0 is not None:
                                mtile = mask_tile(j, self.qb, m0, m1)
                                lo = idx * QB + m0
                                hi = idx * QB + m1
                                nc.vector.tensor_mul(
                                    e_sb[:, lo:hi], e_sb[:, lo:hi],
                                    mtile[:, 0:m1 - m0])
                        return e_sb

                    def pv_and_acc(self, grp, g0, e_sb):
                        for idx, (j, q0c, m0, m1) in enumerate(grp):
                            gi = g0 + idx
                            nc.tensor.matmul(
                                self.o_ps[:, q0c:QB],
                                v_sb[:, (self.b * NKT + j) * 256 + self.h * 128:
                                     (self.b * NKT + j) * 256 + (self.h + 1) * 128],
                                e_sb[:, idx * QB + q0c:(idx + 1) * QB],
                                start=(gi == 0), stop=(gi == self.ne - 1))
                            use_g = gi % 3 == 2  # GPSIMD chain (slower)
                            eng = nc.gpsimd if use_g else nc.vector
                            acc = self.accA if use_g else self.accB
                            st = self.startA if use_g else self.startB
                            esl = e_sb[:, idx * QB + q0c:(idx + 1) * QB]
                            if st is None:
                                eng.tensor_copy(acc[:, q0c:QB], esl)
                                st = q0c
                            elif q0c < st:
                                eng.tensor_copy(acc[:, q0c:st],
                                                esl[:, 0:st - q0c])
                                eng.tensor_add(acc[:, st:QB], acc[:, st:QB],
                                               esl[:, st - q0c:])
                                st = q0c
                            else:
                                eng.tensor_add(acc[:, q0c:QB], acc[:, q0c:QB],
                                               esl)
                            if use_g:
                                self.startA = st
                            else:
                                self.startB = st

                    def step(self):
                        # one group of 2 entries: S+exp for group at gi,
                        # PV for the previous group
                        if self.gi < self.ne:
                            grp = self.entries[self.gi:self.gi + 2]
                            e_sb = self.s_and_exp(grp)
                            if self.pend is not None:
                                self.pv_and_acc(*self.pend)
                            self.pend = (grp, self.gi, e_sb)
                            self.gi += len(grp)
                            return True
                        return False

                    def finish(self):
                        if self.pend is not None:
                            self.pv_and_acc(*self.pend)
                            self.pend = None
                        if self.startA is not None:
                            sa = max(self.startA, self.startB)
                            if self.startA < self.startB:
                                # accA covers a wider prefix: swap roles
                                nc.vector.tensor_add(
                                    self.accA[:, sa:QB], self.accA[:, sa:QB],
                                    self.accB[:, sa:QB])
                                merged = self.accA
                            else:
                                nc.vector.tensor_add(
                                    self.accB[:, sa:QB], self.accB[:, sa:QB],
                                    self.accA[:, sa:QB])
                                merged = self.accB
                        else:
                            merged = self.accB
                        db_ps = y_ps_pool.tile([128, QB], f32, tag="y",
                                               name="dbps")
                        nc.tensor.matmul(db_ps[0:1, :], ones_col[:],
                                         merged[:], start=True, stop=True)
                        rcp = rcp_pool.tile([1, QB], f32r, tag="rcp",
                                            name="rcp")
                        nc.vector.reciprocal(rcp[:], db_ps[0:1, :])
                        nc.tensor.matmul(db_ps[:], ones_row[:], rcp[:],
                                         start=True, stop=True)
                        b_sb = b_pool.tile([128, QB], f32, tag="bsb",
                                           name="bsb")
                        nc.scalar.copy(b_sb[:], db_ps[:])
                        o_sb = b_pool.tile([128, QB], f32, tag="osb",
                                           name="osb")
                        nc.scalar.copy(o_sb[:], self.o_ps[:])
                        nc.vector.tensor_mul(
                            at_sb[self.h][:, self.qcol:self.qcol + QB],
                            o_sb[:], b_sb[:])

                def emit_proj_tile(b, tt):
                    trow = (b * NKT + tt) * 128
                    for ch in range(4):
                        y_ps = y_ps_pool.tile([128, 512], f32, tag="y",
                                              name="yps")
                        for hh in range(HPC):
                            nc.tensor.matmul(
                                y_ps[:],
                                at_sb[hh][:, trow:trow + 128],
                                wo_sb[hh][:, ch * 512:(ch + 1) * 512],
                                start=(hh == 0), stop=(hh == HPC - 1))
                        y_sb = ysb_pool.tile([128, 512], f32, tag="ysb",
                                             name="ysb")
                        if ch % 2 == 0:
                            nc.scalar.copy(y_sb[:], y_ps[:])
                        else:
                            nc.vector.tensor_copy(y_sb[:], y_ps[:])
                        nc.sync.dma_start(
                            y_d.ap()[trow:trow + 128,
                                     ch * 512:(ch + 1) * 512], y_sb[:])

                proj_queue = []  # (b, tt) pending projection tiles

                def drain_proj(n):
                    for _ in range(min(n, len(proj_queue))):
                        emit_proj_tile(*proj_queue.pop(0))

                for b in range(B):
                    for qb in range(NQB):
                        streams = [QbStream(b, h, qb) for h in range(HPC)]
                        alive = True
                        nstep = 0
                        while alive:
                            alive = False
                            for st in streams:
                                if st.step():
                                    alive = True
                            nstep += 1
                            drain_proj(1)
                        for st in streams:
                            st.finish()
                        proj_queue.extend((b, qb * 4 + t4) for t4 in range(4))
                drain_proj(len(proj_queue))

            if True:
                if dump:
                    dsts = [qt_sb[0], qt_sb[1], kt_sb[0], kt_sb[1]]
                    for e in range(4):
                        nc.sync.dma_start(
                            qk_dump.ap()[e * 128:(e + 1) * 128, :], dsts[e][:])
                    nc.sync.dma_start(v_dump.ap()[:, :], v_sb[:])
                    for h in range(HPC):
                        nc.sync.dma_start(
                            at_dump.ap()[h * 128:(h + 1) * 128, :], at_sb[h][:])
    nc.compile()
    return nc


last_results = None  # set when KERNEL_TRACE=1 (profiling from test harness)


def kernel(x, mask, W_qkv, W_o):
    import os
    from concourse.bass_utils import run_bass_kernel_spmd

    x = np.asarray(x, dtype=np.float32)
    mask_np = np.asarray(mask).astype(bool)
    W_qkv = np.asarray(W_qkv, dtype=np.float32)
    W_o = np.asarray(W_o, dtype=np.float32)
    m2 = np.broadcast_to(mask_np, (1, 1, S, S))[0, 0]

    key = m2.tobytes()
    nc = _cache.get(key)
    if nc is None:
        nc = _build(m2)
        _cache[key] = nc

    xt = np.ascontiguousarray(x.reshape(T, D).T)            # [D, T]
    mt = np.ascontiguousarray(m2.T.astype(np.float32))      # [k, q]

    in_maps = []
    for c in range(NCORES):
        hA, hB = HPC * c, HPC * c + 1
        q_rows = list(range(hA * DK, (hA + 1) * DK)) + \
                 list(range(hB * DK, (hB + 1) * DK))
        k_rows = [D + r for r in q_rows]
        v_rows = [2 * D + r for r in q_rows]
        wqk = np.ascontiguousarray(W_qkv[q_rows + k_rows, :].T)  # [D, 512]
        wv = np.ascontiguousarray(W_qkv[v_rows, :].T)            # [D, 256]
        wo = np.ascontiguousarray(W_o[:, q_rows].T)              # [256, D]
        in_maps.append({"xt": xt, "wqk": wqk, "wv": wv, "wo": wo, "mt": mt})

    trace = bool(os.environ.get("KERNEL_TRACE"))
    res = run_bass_kernel_spmd(nc, in_maps, core_ids=list(range(NCORES)),
                               trace=trace)
    if trace:
        global last_results
        last_results = res
    y = res.results[0]["y"].copy()
    for c in range(1, NCORES):
        y += res.results[c]["y"]
    return y.reshape(B, S, D)



# revision 5
# speedup vs baseline: 1.2337x; 1.2337x over previous
"""Multi-head causal attention on 8 Trainium2 NeuronCores - v2.

Tensor-parallel over heads (2/core). Single interleaved PE instruction
stream: QKV projection chunk waves, attention units and out-projection
units are emitted into one dense sequence so the PE never idles, HAM
stays warm, and ACT/DVE softmax work hides under matmuls.

Data is bf16 (inputs, Q/K/V, exp(S), attention out, W_o, y partials);
PSUM accumulation stays f32. Softmax denominator is computed on the PE:
db[128,512] += ones128.T @ e per entry (clean pairs pre-summed on DVE
at 2x bf16 rate), so no partition-reduce chains and no [1,512] ops;
1/d via reciprocal_approx_fast on [128,512], one DVE multiply
(o_ps PSUM operand) produces the normalized attention out.

PSUM budget (8 banks): qk 2 + v 1 + s 2 + o 1 + db 1 + y 1.
"""
import sys
if '/opt/trn_rl_repo' not in sys.path:
    sys.path.insert(0, '/opt/trn_rl_repo')

import numpy as np

B, S, D = 2, 2048, 2048
H, DK = 16, 128
NCORES = 8
HPC = H // NCORES            # heads per core
T = B * S                    # tokens
QB = 512                     # q-block width
NKT = S // 128               # k tiles per batch (16)
NQB = S // QB                # q blocks per batch (4)
NCH = T // QB                # token chunks (8)
NDT = D // 128               # d_model tiles (16)

_cache = {}


def _analyze_mask(m2):
    """m2: [S, S] bool. Returns blocks[qb] = list of entries
    (j, q0, m0, m1) ascending j: q0 first valid col (block-local),
    m0..m1 mask-multiply range (None if fully valid from q0)."""
    blocks = []
    for qb in range(NQB):
        entries = []
        for j in range(NKT):
            blk = m2[qb * QB:(qb + 1) * QB, j * 128:(j + 1) * 128]
            col_any = blk.any(axis=1)
            if not col_any.any():
                continue
            col_all = blk.all(axis=1)
            q0 = int(np.argmax(col_any))
            rev = col_all[::-1]
            run = int(np.argmin(rev)) if not rev.all() else QB
            q1 = QB - run
            if q1 <= q0:
                entries.append((j, q0, None, None))
            else:
                entries.append((j, q0, q0, q1))
        blocks.append(entries)
    return blocks


def _build(mask_bool):
    from contextlib import ExitStack
    import concourse.bass as bass
    import concourse.tile as tile
    from concourse import bacc, mybir

    f32 = mybir.dt.float32
    f32r = mybir.dt.float32r
    bf16 = mybir.dt.bfloat16
    EXP = mybir.ActivationFunctionType.Exp
    scale = 1.0 / np.sqrt(DK)

    m2 = mask_bool
    blocks = _analyze_mask(m2)
    for ents in blocks:
        assert ents and ents[0][1] == min(e[1] for e in ents), \
            "first entry must cover the widest q range"

    nc = bacc.Bacc("TRN2", target_bir_lowering=False, debug=False)
    xt_d = nc.dram_tensor("xt", [D, T], bf16, kind="ExternalInput")
    wqk_d = nc.dram_tensor("wqk", [D, 4 * 128], bf16, kind="ExternalInput")
    wv_d = nc.dram_tensor("wv", [D, 2 * 128], bf16, kind="ExternalInput")
    wo_d = nc.dram_tensor("wo", [2 * 128, D], bf16, kind="ExternalInput")
    mt_d = nc.dram_tensor("mt", [S, S], bf16, kind="ExternalInput")
    y_d = nc.dram_tensor("y", [T, D], bf16, kind="ExternalOutput")
    import os as _os
    dump = bool(_os.environ.get("KERNEL_DUMP"))
    if dump:
        qk_dump = nc.dram_tensor("qk_dump", [512, T], bf16,
                                 kind="ExternalOutput")
        v_dump = nc.dram_tensor("v_dump", [128, (T // 128) * 256], bf16,
                                kind="ExternalOutput")
        at_dump = nc.dram_tensor("at_dump", [256, T], bf16,
                                 kind="ExternalOutput")
        d_dump = nc.dram_tensor("d_dump", [128, 512], f32,
                                kind="ExternalOutput")

    with tile.TileContext(nc) as tc:
        with ExitStack() as stack:
            stack.enter_context(
                nc.allow_low_precision(reason="bf16 kernel"))
            qkt_pool = stack.enter_context(tc.tile_pool(name="qkt", bufs=1))
            v_pool = stack.enter_context(tc.tile_pool(name="vsb", bufs=1))
            att_pool = stack.enter_context(tc.tile_pool(name="att", bufs=1))
            cst_pool = stack.enter_context(tc.tile_pool(name="cst", bufs=1))
            w_pool = stack.enter_context(tc.tile_pool(name="wts", bufs=1))
            xt_pool = stack.enter_context(tc.tile_pool(name="xt", bufs=32))
            e_pool = stack.enter_context(tc.tile_pool(name="e", bufs=6))
            es_pool = stack.enter_context(tc.tile_pool(name="es", bufs=4))
            rcp_pool = stack.enter_context(tc.tile_pool(name="rcp", bufs=2))
            msk_pool = stack.enter_context(tc.tile_pool(name="msk", bufs=1))
            ysb_pool = stack.enter_context(tc.tile_pool(name="ysb", bufs=4))

            qk_ps_pool = stack.enter_context(
                tc.tile_pool(name="ps_qk", bufs=1, space="PSUM"))
            v_ps_pool = stack.enter_context(
                tc.tile_pool(name="ps_v", bufs=2, space="PSUM"))
            s_ps_pool = stack.enter_context(
                tc.tile_pool(name="ps_s", bufs=1, space="PSUM"))
            o_ps_pool = stack.enter_context(
                tc.tile_pool(name="ps_o", bufs=1, space="PSUM"))
            db_ps_pool = stack.enter_context(
                tc.tile_pool(name="ps_db", bufs=1, space="PSUM"))
            y_ps_pool = stack.enter_context(
                tc.tile_pool(name="ps_y", bufs=1, space="PSUM"))

            # ------------ persistent SBUF ------------
            qt_sb = [qkt_pool.tile([128, T], bf16, tag=f"qt{h}", name=f"qt{h}")
                     for h in range(HPC)]
            kt_sb = [qkt_pool.tile([128, T], bf16, tag=f"kt{h}", name=f"kt{h}")
                     for h in range(HPC)]
            v_sb = v_pool.tile([128, (T // 128) * 256], bf16, tag="v")
            at_sb = [att_pool.tile([128, T], bf16, tag=f"at{h}", name=f"at{h}")
                     for h in range(HPC)]

            ones_f = cst_pool.tile([128, 128], f32, tag="ones_f")
            nc.vector.memset(ones_f[:], 1.0)
            ones_bf = cst_pool.tile([128, 128], bf16, tag="ones_bf")
            nc.scalar.copy(ones_bf[:], ones_f[:])
            # warm up the exp table early (ACT_TABLE_LOAD ~2.7us)
            exp_warm = cst_pool.tile([1, 1], f32, tag="expw")
            nc.scalar.activation(exp_warm[:], ones_f[0:1, 0:1], EXP)

            # ------------ weights: DMA interleaved with first chunk ------
            wqk_sb, wv_sb, wo_sb = [], [], []
            for kd in range(NDT):
                wq = w_pool.tile([128, 512], bf16, tag=f"wqk{kd}")
                wqk_sb.append(wq)
                wv_t = w_pool.tile([128, 256], bf16, tag=f"wv{kd}")
                wv_sb.append(wv_t)
            for h in range(HPC):
                wt = w_pool.tile([128, D], bf16, tag=f"wo{h}", name=f"wo{h}")
                wo_sb.append(wt)

            # xt tiles: one [128, 1024] per (chunk-pair, kd)
            xt_tiles = {}

            def xt_tile(pair, kd):
                t = xt_tiles.get((pair, kd))
                if t is None:
                    t = xt_pool.tile([128, 1024], bf16, tag="xt")
                    nc.sync.dma_start(
                        t[:], xt_d.ap()[kd * 128:(kd + 1) * 128,
                                        pair * 1024:(pair + 1) * 1024])
                    xt_tiles[(pair, kd)] = t
                return t

            # mask tile cache keyed by content
            mask_tiles = {}

            def mask_tile(j, qb, m0, m1):
                key = m2[qb * QB + m0:qb * QB + m1,
                         j * 128:(j + 1) * 128].tobytes()
                t = mask_tiles.get(key)
                if t is None:
                    t = msk_pool.tile([128, QB], bf16,
                                      name=f"mask{len(mask_tiles)}",
                                      tag=f"m{len(mask_tiles)}")
                    nc.sync.dma_start(
                        t[:, 0:m1 - m0],
                        mt_d.ap()[j * 128:(j + 1) * 128,
                                  qb * QB + m0:qb * QB + m1])
                    mask_tiles[key] = t
                return t

            # ---------------- attention stream ----------------
            class Stream:
                """One (b, h, qb): units alternate S/exp and d/PV."""

                def __init__(self, b, h, qb):
                    self.b, self.h, self.qb = b, h, qb
                    self.tb = b * S
                    ents = blocks[qb]
                    # pair consecutive entries; a pair is "clean" if both
                    # are fully valid from col 0 (esum fast path)
                    self.groups = []
                    i = 0
                    while i < len(ents):
                        grp = ents[i:i + 2]
                        clean = (len(grp) == 2 and
                                 all(g[1] == 0 and g[2] is None
                                     for g in grp))
                        self.groups.append((grp, i, clean))
                        i += len(grp)
                    self.ne = len(ents)
                    self.qcol = self.tb + qb * QB
                    self.o_ps = None
                    self.db_ps = None
                    self.gi = 0
                    self.pend = None
                    self.done_units = False

                def unit_s(self, grp, clean):
                    """S matmuls + exp (+ mask) for one group."""
                    h, tb = self.h, self.tb
                    s_t = s_ps_pool.tile([128, 2 * QB], f32, tag="s",
                                         name="sps")
                    for idx, (j, q0, m0, m1) in enumerate(grp):
                        nc.tensor.matmul(
                            s_t[:, idx * QB + q0:(idx + 1) * QB],
                            kt_sb[h][:, tb + j * 128:tb + (j + 1) * 128],
                            qt_sb[h][:, self.qcol + q0:self.qcol + QB],
                            start=True, stop=True)
                    e_t = e_pool.tile([128, 2 * QB], bf16, tag="e",
                                      name="esb")
                    if clean:
                        nc.scalar.activation(e_t[:], s_t[:], EXP,
                                             scale=scale)
                    else:
                        for idx, (j, q0, m0, m1) in enumerate(grp):
                            lo = idx * QB + q0
                            hi = (idx + 1) * QB
                            nc.scalar.activation(
                                e_t[:, lo:hi], s_t[:, lo:hi], EXP,
                                scale=scale)
                    for idx, (j, q0, m0, m1) in enumerate(grp):
                        if m0 is not None:
                            mt = mask_tile(j, self.qb, m0, m1)
                            lo = idx * QB + m0
                            hi = idx * QB + m1
                            nc.vector.tensor_mul(
                                e_t[:, lo:hi], e_t[:, lo:hi],
                                mt[:, 0:m1 - m0])
                    if clean:
                        es_t = es_pool.tile([128, QB], bf16, tag="es",
                                            name="essb")
                        nc.vector.tensor_add(es_t[:], e_t[:, 0:QB],
                                             e_t[:, QB:2 * QB])
                    else:
                        es_t = None
                    return (grp, clean, e_t, es_t)

                def unit_pv(self, pend, g0):
                    """d matmuls + PV matmuls for a completed group."""
                    grp, clean, e_t, es_t = pend
                    b, h = self.b, self.h
                    if self.o_ps is None:
                        self.o_ps = o_ps_pool.tile([128, QB], f32,
                                                   tag="o", name="ops")
                        self.db_ps = db_ps_pool.tile([128, QB], f32,
                                                     tag="db", name="dbps")
                    first = g0 == 0
                    last_d = g0 + len(grp) == self.ne
                    if clean:
                        nc.tensor.matmul(
                            self.db_ps[:], ones_bf[:], es_t[:],
                            start=first, stop=last_d)
                    else:
                        for idx, (j, q0, m0, m1) in enumerate(grp):
                            nc.tensor.matmul(
                                self.db_ps[:, q0:QB], ones_bf[:],
                                e_t[:, idx * QB + q0:(idx + 1) * QB],
                                start=first and idx == 0,
                                stop=g0 + idx == self.ne - 1)
                    for idx, (j, q0, m0, m1) in enumerate(grp):
                        gi = g0 + idx
                        nc.tensor.matmul(
                            self.o_ps[:, q0:QB],
                            v_sb[:, (b * NKT + j) * 256 + h * 128:
                                 (b * NKT + j) * 256 + (h + 1) * 128],
                            e_t[:, idx * QB + q0:(idx + 1) * QB],
                            start=(gi == 0), stop=(gi == self.ne - 1))

                def unit_tail(self):
                    import os as _os
                    if dump and self.b == 0 and self.h == 0 and self.qb == 0:
                        dtmp = rcp_pool.tile([128, QB], f32, tag="dtmp",
                                             name="dtmp")
                        nc.vector.tensor_copy(dtmp[:], self.db_ps[:])
                        nc.sync.dma_start(d_dump.ap()[:, :], dtmp[:])
                    rcp = rcp_pool.tile([128, QB], f32, tag="rcp",
                                        name="rcp")
                    if _os.environ.get("SLOW_RECIP"):
                        nc.vector.reciprocal(rcp[:], self.db_ps[:])
                    else:
                        nc.vector.reciprocal_approx_fast(rcp[:],
                                                         self.db_ps[:])
                    nc.vector.tensor_mul(
                        at_sb[self.h][:, self.qcol:self.qcol + QB],
                        self.o_ps[:], rcp[:])

                def step(self):
                    """Emit one unit. Returns False when stream done."""
                    if self.gi < len(self.groups):
                        grp, g0, clean = self.groups[self.gi]
                        nxt = self.unit_s(grp, clean)
                        if self.pend is not None:
                            self.unit_pv(self.pend[0], self.pend[1])
                        self.pend = (nxt, g0)
                        self.gi += 1
                        return True
                    if self.pend is not None:
                        self.unit_pv(self.pend[0], self.pend[1])
                        self.pend = None
                        return True
                    if not self.done_units:
                        self.unit_tail()
                        self.done_units = True
                        return True
                    return False

            # ---------------- projection units ----------------
            proj_queue = []   # (b, tt, ch)

            def emit_proj_unit():
                b, tt, ch = proj_queue.pop(0)
                trow = b * S + tt * 128
                y_ps = y_ps_pool.tile([128, 512], f32, tag="y", name="yps")
                for hh in range(HPC):
                    nc.tensor.matmul(
                        y_ps[:],
                        at_sb[hh][:, trow:trow + 128],
                        wo_sb[hh][:, ch * 512:(ch + 1) * 512],
                        start=(hh == 0), stop=(hh == HPC - 1))
                y_sb = ysb_pool.tile([128, 512], bf16, tag="ysb",
                                     name="ysb")
                if (tt + ch) % 2 == 0:
                    nc.scalar.copy(y_sb[:], y_ps[:])
                else:
                    nc.vector.tensor_copy(y_sb[:], y_ps[:])
                nc.sync.dma_start(
                    y_d.ap()[trow:trow + 128, ch * 512:(ch + 1) * 512],
                    y_sb[:])

            # ---------------- scheduler ----------------
            stream_list = []    # in ready order with gates
            for b in range(B):
                for qb in range(NQB):
                    for h in range(HPC):
                        stream_list.append((4 * b + qb, Stream(b, h, qb)))
            sched = {'si': 0, 'chunks_done': 0, 'tog': False}

            def cur_stream():
                si = sched['si']
                if si >= len(stream_list):
                    return None
                gate, st = stream_list[si]
                if gate >= sched['chunks_done']:
                    return None
                return st

            def step_stream():
                st = cur_stream()
                while st is not None:
                    if st.step():
                        return True
                    # stream finished: queue its projection, advance
                    si = sched['si']
                    _, stt = stream_list[si]
                    if stt.h == HPC - 1:
                        for t4 in range(4):
                            for ch in range(4):
                                proj_queue.append(
                                    (stt.b, stt.qb * 4 + t4, ch))
                    sched['si'] = si + 1
                    st = cur_stream()
                return False

            def fill_slot():
                # alternate stream units and projection units
                tog = sched['tog']
                sched['tog'] = not tog
                if tog and proj_queue:
                    emit_proj_unit()
                    return True
                if step_stream():
                    return True
                if proj_queue:
                    emit_proj_unit()
                    return True
                return False

            # ---------------- phase 1 chunk waves + slots ----------------
            for c in range(NCH):
                pair = c // 2
                half = (c % 2) * 512
                # qk waves: one e-tile each (0,1 = q_h0,q_h1; 2,3 = k_h0,k_h1)
                dsts = [qt_sb[0], qt_sb[1], kt_sb[0], kt_sb[1]]
                for e in range(4):
                    qk_ps = qk_ps_pool.tile([128, 512], f32, tag="qk",
                                            name="qkps")
                    for kd in range(NDT):
                        if c == 0 and e == 0:
                            # interleave weight DMAs with first chunk
                            nc.sync.dma_start(
                                wqk_sb[kd][:],
                                wqk_d.ap()[kd * 128:(kd + 1) * 128, :])
                            nc.sync.dma_start(
                                wv_sb[kd][:],
                                wv_d.ap()[kd * 128:(kd + 1) * 128, :])
                        if c == 0 and e == 1 and kd < 2:
                            nc.sync.dma_start(
                                wo_sb[kd][:],
                                wo_d.ap()[kd * 128:(kd + 1) * 128, :])
                        xt_t = xt_tile(pair, kd)
                        nc.tensor.matmul(
                            qk_ps[:],
                            wqk_sb[kd][:, e * 128:(e + 1) * 128],
                            xt_t[:, half:half + 512],
                            start=kd == 0, stop=kd == NDT - 1)
                        if kd % 4 == 1:
                            fill_slot()
                    nc.vector.tensor_copy(
                        dsts[e][:, c * 512:(c + 1) * 512], qk_ps[:])
                # v waves: 2 token-tiles per wave, one PSUM bank each
                for wave in range(2):
                    v_ps = [v_ps_pool.tile([128, 256], f32, tag="v",
                                           name="vps")
                            for _ in range(2)]
                    for kd in range(NDT):
                        xt_t = xt_tile(pair, kd)
                        st, sp = kd == 0, kd == NDT - 1
                        for t2 in range(2):
                            tl = wave * 2 + t2
                            nc.tensor.matmul(
                                v_ps[t2][:],
                                xt_t[:, half + tl * 128:
                                     half + (tl + 1) * 128],
                                wv_sb[kd][:], start=st, stop=sp)
                        if kd % 2 == 1:
                            fill_slot()
                    for t2 in range(2):
                        tok = c * 4 + wave * 2 + t2
                        nc.scalar.copy(
                            v_sb[:, tok * 256:(tok + 1) * 256], v_ps[t2][:])
                sched['chunks_done'] = c + 1
                # prefetch next pair's xt while this chunk's slots run
                if c % 2 == 1 and c + 1 < NCH:
                    for kd in range(NDT):
                        xt_tile(c // 2 + 1, kd)

            # ---------------- drain: attention + projection ----------------
            while fill_slot():
                pass

            if dump:
                dsts = [qt_sb[0], qt_sb[1], kt_sb[0], kt_sb[1]]
                for e in range(4):
                    nc.sync.dma_start(
                        qk_dump.ap()[e * 128:(e + 1) * 128, :], dsts[e][:])
                nc.sync.dma_start(v_dump.ap()[:, :], v_sb[:])
                for h in range(HPC):
                    nc.sync.dma_start(
                        at_dump.ap()[h * 128:(h + 1) * 128, :], at_sb[h][:])

    nc.compile()
    return nc


last_results = None  # set when KERNEL_TRACE=1


def kernel(x, mask, W_qkv, W_o):
    import os
    import ml_dtypes
    from concourse.bass_utils import run_bass_kernel_spmd

    bf = ml_dtypes.bfloat16
    x = np.asarray(x, dtype=np.float32)
    mask_np = np.asarray(mask).astype(bool)
    W_qkv = np.asarray(W_qkv, dtype=np.float32)
    W_o = np.asarray(W_o, dtype=np.float32)
    m2 = np.broadcast_to(mask_np, (1, 1, S, S))[0, 0]

    key = m2.tobytes()
    nc = _cache.get(key)
    if nc is None:
        nc = _build(m2)
        _cache[key] = nc

    xt = np.ascontiguousarray(x.reshape(T, D).T).astype(bf)     # [D, T]
    mt = np.ascontiguousarray(m2.T.astype(np.float32)).astype(bf)

    in_maps = []
    for c in range(NCORES):
        hA, hB = HPC * c, HPC * c + 1
        q_rows = list(range(hA * DK, (hA + 1) * DK)) + \
                 list(range(hB * DK, (hB + 1) * DK))
        k_rows = [D + r for r in q_rows]
        v_rows = [2 * D + r for r in q_rows]
        wqk = np.ascontiguousarray(W_qkv[q_rows + k_rows, :].T).astype(bf)
        wv = np.ascontiguousarray(W_qkv[v_rows, :].T).astype(bf)
        wo = np.ascontiguousarray(W_o[:, q_rows].T).astype(bf)
        in_maps.append({"xt": xt, "wqk": wqk, "wv": wv, "wo": wo, "mt": mt})

    trace = bool(os.environ.get("KERNEL_TRACE"))
    res = run_bass_kernel_spmd(nc, in_maps, core_ids=list(range(NCORES)),
                               trace=trace)
    if trace:
        global last_results
        last_results = res
    y = res.results[0]["y"].astype(np.float32)
    for c in range(1, NCORES):
        y += res.results[c]["y"].astype(np.float32)
    return y.reshape(B, S, D)


# revision 6
# speedup vs baseline: 1.3493x; 1.0937x over previous
"""Multi-head causal attention on 8 Trainium2 NeuronCores - v2.

Tensor-parallel over heads (2/core). Single interleaved PE instruction
stream: QKV projection chunk waves, attention units and out-projection
units are emitted into one dense sequence so the PE never idles, HAM
stays warm, and ACT/DVE softmax work hides under matmuls.

Data is bf16 (inputs, Q/K/V, exp(S), attention out, W_o, y partials);
PSUM accumulation stays f32. Softmax denominator is computed on the PE:
db[128,512] += ones128.T @ e per entry (clean pairs pre-summed on DVE
at 2x bf16 rate), so no partition-reduce chains and no [1,512] ops;
1/d via reciprocal_approx_fast on [128,512], one DVE multiply
(o_ps PSUM operand) produces the normalized attention out.

PSUM budget (8 banks): qk 2 + v 1 + s 2 + o 1 + db 1 + y 1.
"""
import sys
if '/opt/trn_rl_repo' not in sys.path:
    sys.path.insert(0, '/opt/trn_rl_repo')

import numpy as np

B, S, D = 2, 2048, 2048
H, DK = 16, 128
NCORES = 8
HPC = H // NCORES            # heads per core
T = B * S                    # tokens
QB = 512                     # q-block width
NKT = S // 128               # k tiles per batch (16)
NQB = S // QB                # q blocks per batch (4)
NCH = T // QB                # token chunks (8)
NDT = D // 128               # d_model tiles (16)

_cache = {}


def _analyze_mask(m2):
    """m2: [S, S] bool. Returns blocks[qb] = list of entries
    (j, q0, m0, m1) ascending j: q0 first valid col (block-local),
    m0..m1 mask-multiply range (None if fully valid from q0)."""
    blocks = []
    for qb in range(NQB):
        entries = []
        for j in range(NKT):
            blk = m2[qb * QB:(qb + 1) * QB, j * 128:(j + 1) * 128]
            col_any = blk.any(axis=1)
            if not col_any.any():
                continue
            col_all = blk.all(axis=1)
            q0 = int(np.argmax(col_any))
            rev = col_all[::-1]
            run = int(np.argmin(rev)) if not rev.all() else QB
            q1 = QB - run
            if q1 <= q0:
                entries.append((j, q0, None, None))
            else:
                entries.append((j, q0, q0, q1))
        blocks.append(entries)
    return blocks


def _build(mask_bool):
    from contextlib import ExitStack
    import concourse.bass as bass
    import concourse.tile as tile
    from concourse import bacc, mybir

    f32 = mybir.dt.float32
    f32r = mybir.dt.float32r
    bf16 = mybir.dt.bfloat16
    EXP = mybir.ActivationFunctionType.Exp
    scale = 1.0 / np.sqrt(DK)

    m2 = mask_bool
    blocks = _analyze_mask(m2)
    for ents in blocks:
        assert ents and ents[0][1] == min(e[1] for e in ents), \
            "first entry must cover the widest q range"

    nc = bacc.Bacc("TRN2", target_bir_lowering=False, debug=False)
    xt_d = nc.dram_tensor("xt", [D, T], bf16, kind="ExternalInput")
    wqk_d = nc.dram_tensor("wqk", [D, 4 * 128], bf16, kind="ExternalInput")
    wv_d = nc.dram_tensor("wv", [D, 2 * 128], bf16, kind="ExternalInput")
    wo_d = nc.dram_tensor("wo", [2 * 128, D], bf16, kind="ExternalInput")
    mt_d = nc.dram_tensor("mt", [S, S], bf16, kind="ExternalInput")
    y_d = nc.dram_tensor("y", [T, D], bf16, kind="ExternalOutput")
    import os as _os
    dump = bool(_os.environ.get("KERNEL_DUMP"))
    if dump:
        qk_dump = nc.dram_tensor("qk_dump", [512, T], bf16,
                                 kind="ExternalOutput")
        v_dump = nc.dram_tensor("v_dump", [128, (T // 128) * 256], bf16,
                                kind="ExternalOutput")
        at_dump = nc.dram_tensor("at_dump", [256, T], bf16,
                                 kind="ExternalOutput")
        d_dump = nc.dram_tensor("d_dump", [128, 512], f32,
                                kind="ExternalOutput")

    with tile.TileContext(nc) as tc:
        with ExitStack() as stack:
            stack.enter_context(
                nc.allow_low_precision(reason="bf16 kernel"))
            qkt_pool = stack.enter_context(tc.tile_pool(name="qkt", bufs=1))
            v_pool = stack.enter_context(tc.tile_pool(name="vsb", bufs=1))
            att_pool = stack.enter_context(tc.tile_pool(name="att", bufs=1))
            cst_pool = stack.enter_context(tc.tile_pool(name="cst", bufs=1))
            w_pool = stack.enter_context(tc.tile_pool(name="wts", bufs=1))
            xt_pool = stack.enter_context(tc.tile_pool(name="xt", bufs=48))
            e_pool = stack.enter_context(tc.tile_pool(name="e", bufs=6))
            es_pool = stack.enter_context(tc.tile_pool(name="es", bufs=6))
            rcp_pool = stack.enter_context(tc.tile_pool(name="rcp", bufs=2))
            msk_pool = stack.enter_context(tc.tile_pool(name="msk", bufs=1))
            ysb_pool = stack.enter_context(tc.tile_pool(name="ysb", bufs=4))

            s_ps_pool = stack.enter_context(
                tc.tile_pool(name="ps_s", bufs=1, space="PSUM"))
            o_ps_pool = stack.enter_context(
                tc.tile_pool(name="ps_o", bufs=1, space="PSUM"))
            db_ps_pool = stack.enter_context(
                tc.tile_pool(name="ps_db", bufs=1, space="PSUM"))
            y_ps_pool = stack.enter_context(
                tc.tile_pool(name="ps_y", bufs=1, space="PSUM"))
            # phase-1 pools opened last so they can close before the drain
            p1 = ExitStack()
            qk_ps_pool = p1.enter_context(
                tc.tile_pool(name="ps_qk", bufs=1, space="PSUM"))
            v_ps_pool = p1.enter_context(
                tc.tile_pool(name="ps_v", bufs=2, space="PSUM"))
            # drain-phase extra pools (opened after phase-1 pools close)
            s_pools = [s_ps_pool]
            y_pools = [y_ps_pool]

            # ------------ persistent SBUF ------------
            qt_sb = [qkt_pool.tile([128, T], bf16, tag=f"qt{h}", name=f"qt{h}")
                     for h in range(HPC)]
            kt_sb = [qkt_pool.tile([128, T], bf16, tag=f"kt{h}", name=f"kt{h}")
                     for h in range(HPC)]
            v_sb = v_pool.tile([128, (T // 128) * 256], bf16, tag="v")
            at_sb = [att_pool.tile([128, T], bf16, tag=f"at{h}", name=f"at{h}")
                     for h in range(HPC)]

            ones_f = cst_pool.tile([128, 128], f32, tag="ones_f")
            nc.vector.memset(ones_f[:], 1.0)
            ones_bf = cst_pool.tile([128, 128], bf16, tag="ones_bf")
            nc.scalar.copy(ones_bf[:], ones_f[:])
            # warm up the exp table early (ACT_TABLE_LOAD ~2.7us)
            exp_warm = cst_pool.tile([1, 1], f32, tag="expw")
            nc.scalar.activation(exp_warm[:], ones_f[0:1, 0:1], EXP)

            # ------------ weights: DMA interleaved with first chunk ------
            wqk_sb, wv_sb, wo_sb = [], [], []
            for kd in range(NDT):
                wq = w_pool.tile([128, 512], bf16, tag=f"wqk{kd}")
                wqk_sb.append(wq)
                wv_t = w_pool.tile([128, 256], bf16, tag=f"wv{kd}")
                wv_sb.append(wv_t)
            for h in range(HPC):
                wt = w_pool.tile([128, D], bf16, tag=f"wo{h}", name=f"wo{h}")
                wo_sb.append(wt)

            # xt tiles: one [128, 512] per (chunk, kd)
            xt_tiles = {}

            def xt_tile(c, kd):
                t = xt_tiles.get((c, kd))
                if t is None:
                    t = xt_pool.tile([128, 512], bf16, tag="xt")
                    nc.sync.dma_start(
                        t[:], xt_d.ap()[kd * 128:(kd + 1) * 128,
                                        c * 512:(c + 1) * 512])
                    xt_tiles[(c, kd)] = t
                return t

            # mask tile cache keyed by content
            mask_tiles = {}

            def mask_tile(j, qb, m0, m1):
                key = m2[qb * QB + m0:qb * QB + m1,
                         j * 128:(j + 1) * 128].tobytes()
                t = mask_tiles.get(key)
                if t is None:
                    t = msk_pool.tile([128, QB], bf16,
                                      name=f"mask{len(mask_tiles)}",
                                      tag=f"m{len(mask_tiles)}")
                    nc.sync.dma_start(
                        t[:, 0:m1 - m0],
                        mt_d.ap()[j * 128:(j + 1) * 128,
                                  qb * QB + m0:qb * QB + m1])
                    mask_tiles[key] = t
                return t

            # ---------------- attention stream ----------------
            class Stream:
                """One (b, h, qb): units alternate S/exp and d/PV."""

                def __init__(self, b, h, qb):
                    self.b, self.h, self.qb = b, h, qb
                    self.tb = b * S
                    ents = blocks[qb]
                    # pair consecutive entries; a pair is "clean" if both
                    # are fully valid from col 0 (esum fast path)
                    self.groups = []
                    i = 0
                    while i < len(ents):
                        grp = ents[i:i + 2]
                        clean = (len(grp) == 2 and
                                 all(g[1] == 0 and g[2] is None
                                     for g in grp))
                        self.groups.append((grp, i, clean))
                        i += len(grp)
                    self.ne = len(ents)
                    self.qcol = self.tb + qb * QB
                    self.o_ps = None
                    self.db_ps = None
                    self.gi = 0
                    self.pend = None
                    self.pend_es = None   # deferred es tile for quad-d
                    self.done_units = False

                def unit_s(self, grp, clean):
                    """S matmuls + exp (+ mask) for one group."""
                    h, tb = self.h, self.tb
                    sp = s_pools[sched['sc'] % len(s_pools)]
                    sched['sc'] += 1
                    s_t = sp.tile([128, 2 * QB], f32, tag="s",
                                  name="sps")
                    for idx, (j, q0, m0, m1) in enumerate(grp):
                        nc.tensor.matmul(
                            s_t[:, idx * QB + q0:(idx + 1) * QB],
                            kt_sb[h][:, tb + j * 128:tb + (j + 1) * 128],
                            qt_sb[h][:, self.qcol + q0:self.qcol + QB],
                            start=True, stop=True)
                    e_t = e_pool.tile([128, 2 * QB], bf16, tag="e",
                                      name="esb")
                    if clean:
                        nc.scalar.activation(e_t[:], s_t[:], EXP,
                                             scale=scale)
                    else:
                        for idx, (j, q0, m0, m1) in enumerate(grp):
                            lo = idx * QB + q0
                            hi = (idx + 1) * QB
                            nc.scalar.activation(
                                e_t[:, lo:hi], s_t[:, lo:hi], EXP,
                                scale=scale)
                    for idx, (j, q0, m0, m1) in enumerate(grp):
                        if m0 is not None:
                            mt = mask_tile(j, self.qb, m0, m1)
                            lo = idx * QB + m0
                            hi = idx * QB + m1
                            nc.vector.tensor_mul(
                                e_t[:, lo:hi], e_t[:, lo:hi],
                                mt[:, 0:m1 - m0])
                    if clean:
                        es_t = es_pool.tile([128, QB], bf16, tag="es",
                                            name="essb")
                        nc.vector.tensor_add(es_t[:], e_t[:, 0:QB],
                                             e_t[:, QB:2 * QB])
                    else:
                        es_t = None
                    return (grp, clean, e_t, es_t)

                def unit_pv(self, pend, g0):
                    """d matmuls + PV matmuls for a completed group."""
                    grp, clean, e_t, es_t = pend
                    b, h = self.b, self.h
                    if self.o_ps is None:
                        self.o_ps = o_ps_pool.tile([128, QB], f32,
                                                   tag="o", name="ops")
                        self.db_ps = db_ps_pool.tile([128, QB], f32,
                                                     tag="db", name="dbps")
                    first = g0 == 0
                    last_d = g0 + len(grp) == self.ne
                    gpos = g0 // 2
                    if clean:
                        nxt_clean = (gpos + 1 < len(self.groups) and
                                     self.groups[gpos + 1][2])
                        if (self.pend_es is None and not last_d
                                and nxt_clean):
                            self.pend_es = (es_t, first)   # defer (quad-d)
                        elif self.pend_es is not None:
                            pes, pfirst = self.pend_es
                            self.pend_es = None
                            es2 = es_pool.tile([128, QB], bf16, tag="es",
                                               name="es2")
                            nc.vector.tensor_add(es2[:], pes[:], es_t[:])
                            nc.tensor.matmul(
                                self.db_ps[:], ones_bf[:], es2[:],
                                start=pfirst, stop=last_d)
                        else:
                            nc.tensor.matmul(
                                self.db_ps[:], ones_bf[:], es_t[:],
                                start=first, stop=last_d)
                    else:
                        if self.pend_es is not None:
                            pes, pfirst = self.pend_es
                            self.pend_es = None
                            nc.tensor.matmul(
                                self.db_ps[:], ones_bf[:], pes[:],
                                start=pfirst, stop=False)
                        for idx, (j, q0, m0, m1) in enumerate(grp):
                            nc.tensor.matmul(
                                self.db_ps[:, q0:QB], ones_bf[:],
                                e_t[:, idx * QB + q0:(idx + 1) * QB],
                                start=first and idx == 0,
                                stop=g0 + idx == self.ne - 1)
                    for idx, (j, q0, m0, m1) in enumerate(grp):
                        gi = g0 + idx
                        nc.tensor.matmul(
                            self.o_ps[:, q0:QB],
                            v_sb[:, (b * NKT + j) * 256 + h * 128:
                                 (b * NKT + j) * 256 + (h + 1) * 128],
                            e_t[:, idx * QB + q0:(idx + 1) * QB],
                            start=(gi == 0), stop=(gi == self.ne - 1))

                def unit_tail(self):
                    import os as _os
                    if dump and self.b == 0 and self.h == 0 and self.qb == 0:
                        dtmp = rcp_pool.tile([128, QB], f32, tag="dtmp",
                                             name="dtmp")
                        nc.vector.tensor_copy(dtmp[:], self.db_ps[:])
                        nc.sync.dma_start(d_dump.ap()[:, :], dtmp[:])
                    rcp = rcp_pool.tile([128, QB], f32, tag="rcp",
                                        name="rcp")
                    if _os.environ.get("SLOW_RECIP"):
                        nc.vector.reciprocal(rcp[:], self.db_ps[:])
                    else:
                        nc.vector.reciprocal_approx_fast(rcp[:],
                                                         self.db_ps[:])
                    nc.vector.tensor_mul(
                        at_sb[self.h][:, self.qcol:self.qcol + QB],
                        self.o_ps[:], rcp[:])

                def step(self):
                    """Emit one unit. Returns False when stream done."""
                    if self.gi < len(self.groups):
                        grp, g0, clean = self.groups[self.gi]
                        nxt = self.unit_s(grp, clean)
                        if self.pend is not None:
                            self.unit_pv(self.pend[0], self.pend[1])
                        self.pend = (nxt, g0)
                        self.gi += 1
                        return True
                    if self.pend is not None:
                        self.unit_pv(self.pend[0], self.pend[1])
                        self.pend = None
                        return True
                    if not self.done_units:
                        self.unit_tail()
                        self.done_units = True
                        return True
                    return False

            # ---------------- projection units ----------------
            proj_queue = []   # (b, tt, ch)

            def emit_proj_unit():
                b, tt, ch = proj_queue.pop(0)
                trow = b * S + tt * 128
                yp = y_pools[sched['yc'] % len(y_pools)]
                sched['yc'] += 1
                y_ps = yp.tile([128, 512], f32, tag="y", name="yps")
                for hh in range(HPC):
                    nc.tensor.matmul(
                        y_ps[:],
                        at_sb[hh][:, trow:trow + 128],
                        wo_sb[hh][:, ch * 512:(ch + 1) * 512],
                        start=(hh == 0), stop=(hh == HPC - 1))
                y_sb = ysb_pool.tile([128, 512], bf16, tag="ysb",
                                     name="ysb")
                if (tt + ch) % 2 == 0:
                    nc.scalar.copy(y_sb[:], y_ps[:])
                else:
                    nc.vector.tensor_copy(y_sb[:], y_ps[:])
                nc.sync.dma_start(
                    y_d.ap()[trow:trow + 128, ch * 512:(ch + 1) * 512],
                    y_sb[:])

            # ---------------- scheduler ----------------
            # chunk emission order interleaves batches so attention work
            # is available from the second chunk onward
            chunk_order = [0, 4, 1, 5, 2, 6, 3, 7]
            stream_list = []    # in ready order with chunk gates
            for qb in range(NQB):
                for b in range(B):
                    for h in range(HPC):
                        stream_list.append((4 * b + qb, Stream(b, h, qb)))
            sched = {'si': 0, 'tog': False, 'sc': 0, 'yc': 0}
            chunks_done = set()

            def cur_stream():
                si = sched['si']
                if si >= len(stream_list):
                    return None
                gate, st = stream_list[si]
                if gate not in chunks_done:
                    return None
                return st

            def step_stream():
                st = cur_stream()
                while st is not None:
                    if st.step():
                        return True
                    # stream finished: queue its projection, advance
                    si = sched['si']
                    _, stt = stream_list[si]
                    if stt.h == HPC - 1:
                        for t4 in range(4):
                            for ch in range(4):
                                proj_queue.append(
                                    (stt.b, stt.qb * 4 + t4, ch))
                    sched['si'] = si + 1
                    st = cur_stream()
                return False

            def fill_slot():
                # alternate stream units and projection units
                tog = sched['tog']
                sched['tog'] = not tog
                if tog and proj_queue:
                    emit_proj_unit()
                    return True
                if step_stream():
                    return True
                if proj_queue:
                    emit_proj_unit()
                    return True
                return False

            # ---------------- phase 1 chunk waves + slots ----------------
            for pi, c in enumerate(chunk_order):
                nxt_c = chunk_order[pi + 1] if pi + 1 < NCH else None
                # qk waves: one e-tile each (0,1 = q_h0,q_h1; 2,3 = k_h0,k_h1)
                dsts = [qt_sb[0], qt_sb[1], kt_sb[0], kt_sb[1]]
                for e in range(4):
                    qk_ps = qk_ps_pool.tile([128, 512], f32, tag="qk",
                                            name="qkps")
                    for kd in range(NDT):
                        if pi == 0 and e == 0:
                            # interleave weight DMAs with first chunk
                            nc.sync.dma_start(
                                wqk_sb[kd][:],
                                wqk_d.ap()[kd * 128:(kd + 1) * 128, :])
                        if pi == 0 and e == 1:
                            nc.sync.dma_start(
                                wv_sb[kd][:],
                                wv_d.ap()[kd * 128:(kd + 1) * 128, :])
                            if kd < 2:
                                nc.sync.dma_start(
                                    wo_sb[kd][:],
                                    wo_d.ap()[kd * 128:(kd + 1) * 128, :])
                        if e == 2 and nxt_c is not None:
                            xt_tile(nxt_c, kd)      # prefetch next chunk
                        xt_t = xt_tile(c, kd)
                        nc.tensor.matmul(
                            qk_ps[:],
                            wqk_sb[kd][:, e * 128:(e + 1) * 128],
                            xt_t[:],
                            start=kd == 0, stop=kd == NDT - 1)
                        if kd % 4 == 1:
                            fill_slot()
                    nc.vector.tensor_copy(
                        dsts[e][:, c * 512:(c + 1) * 512], qk_ps[:])
                # v waves: 2 token-tiles per wave, one PSUM bank each
                for wave in range(2):
                    v_ps = [v_ps_pool.tile([128, 256], f32, tag="v",
                                           name="vps")
                            for _ in range(2)]
                    for kd in range(NDT):
                        xt_t = xt_tile(c, kd)
                        st, sp = kd == 0, kd == NDT - 1
                        for t2 in range(2):
                            tl = wave * 2 + t2
                            nc.tensor.matmul(
                                v_ps[t2][:],
                                xt_t[:, tl * 128:(tl + 1) * 128],
                                wv_sb[kd][:], start=st, stop=sp)
                        if kd % 2 == 1:
                            fill_slot()
                    for t2 in range(2):
                        tok = c * 4 + wave * 2 + t2
                        nc.scalar.copy(
                            v_sb[:, tok * 256:(tok + 1) * 256], v_ps[t2][:])
                chunks_done.add(c)

            # ---------------- drain: attention + projection ----------------
            # phase-1 PSUM banks freed -> extra s/y pools for pipelining
            p1.close()
            s2_pool = stack.enter_context(
                tc.tile_pool(name="ps_s2", bufs=1, space="PSUM"))
            y2_pool = stack.enter_context(
                tc.tile_pool(name="ps_y2", bufs=1, space="PSUM"))
            s_pools.append(s2_pool)
            y_pools.append(y2_pool)
            while fill_slot():
                pass

            if dump:
                dsts = [qt_sb[0], qt_sb[1], kt_sb[0], kt_sb[1]]
                for e in range(4):
                    nc.sync.dma_start(
                        qk_dump.ap()[e * 128:(e + 1) * 128, :], dsts[e][:])
                nc.sync.dma_start(v_dump.ap()[:, :], v_sb[:])
                for h in range(HPC):
                    nc.sync.dma_start(
                        at_dump.ap()[h * 128:(h + 1) * 128, :], at_sb[h][:])

    nc.compile()
    return nc


last_results = None  # set when KERNEL_TRACE=1


def kernel(x, mask, W_qkv, W_o):
    import os
    import ml_dtypes
    from concourse.bass_utils import run_bass_kernel_spmd

    bf = ml_dtypes.bfloat16
    x = np.asarray(x, dtype=np.float32)
    mask_np = np.asarray(mask).astype(bool)
    W_qkv = np.asarray(W_qkv, dtype=np.float32)
    W_o = np.asarray(W_o, dtype=np.float32)
    m2 = np.broadcast_to(mask_np, (1, 1, S, S))[0, 0]

    key = m2.tobytes()
    nc = _cache.get(key)
    if nc is None:
        nc = _build(m2)
        _cache[key] = nc

    xt = np.ascontiguousarray(x.reshape(T, D).T).astype(bf)     # [D, T]
    mt = np.ascontiguousarray(m2.T.astype(np.float32)).astype(bf)

    in_maps = []
    for c in range(NCORES):
        hA, hB = HPC * c, HPC * c + 1
        q_rows = list(range(hA * DK, (hA + 1) * DK)) + \
                 list(range(hB * DK, (hB + 1) * DK))
        k_rows = [D + r for r in q_rows]
        v_rows = [2 * D + r for r in q_rows]
        wqk = np.ascontiguousarray(W_qkv[q_rows + k_rows, :].T).astype(bf)
        wv = np.ascontiguousarray(W_qkv[v_rows, :].T).astype(bf)
        wo = np.ascontiguousarray(W_o[:, q_rows].T).astype(bf)
        in_maps.append({"xt": xt, "wqk": wqk, "wv": wv, "wo": wo, "mt": mt})

    trace = bool(os.environ.get("KERNEL_TRACE"))
    res = run_bass_kernel_spmd(nc, in_maps, core_ids=list(range(NCORES)),
                               trace=trace)
    if trace:
        global last_results
        last_results = res
    y = res.results[0]["y"].astype(np.float32)
    for c in range(1, NCORES):
        y += res.results[c]["y"].astype(np.float32)
    return y.reshape(B, S, D)
